# revision 1
# baseline (speedup 1.0000x reference)
"""MoE ExpertFeedForward (top-k routing + per-expert SwiGLU) on 8 Trainium2
NeuronCores via expert parallelism.

Host: router (logits/top-k/softmax), token dispatch, weighted combine.
Device (one expert per core): X.T laid out [D, C] column-major tokens;
  G = silu(Wg @ x), V = W1 @ x, Z = G*V, out.T = W2 @ Z,
  all as 128x128-stationary matmuls streaming token columns.
"""

import numpy as np

D_MODEL = 1024
D_FF = 4096
N_EXPERTS = 8
N_CORES = 8
P = 128
TW = 512  # token-tile width (matmul moving free dim / one PSUM bank of fp32)
DK = D_MODEL // P   # 8 contraction tiles for MM1/MM2
FI = D_FF // P      # 32 f tiles
DO = D_MODEL // P   # 8 output-d tiles for MM3

USE_BF16 = True

# ---------------------------------------------------------------------------
# BIR post-pass: TRN2 instruction encoding has ONE sync-wait slot; Tile can
# emit several waits on one instruction, which this walrus build rejects
# ("Too many sync wait commands").  Peel extra waits onto single-wait NoOps
# inserted just before the instruction on the same engine.
# ---------------------------------------------------------------------------

_bir_fix_installed = False


def _fix_bir_bytes(bir):
    import orjson

    m = orjson.loads(bir)
    changed = False
    for fn in m.get("functions", []):
        for blk in fn.get("blocks", []):
            out = []
            for inst in blk.get("instructions", []):
                si = inst.get("sync_info")
                if si:
                    waits = si.get("on_wait") or []
                    upds = si.get("on_update") or []
                    if len(waits) > 1:
                        changed = True
                        for k, w in enumerate(waits[:-1]):
                            out.append({
                                "name": f"{inst['name']}_pw{k}",
                                "opcode": "NoOp",
                                "engine": inst.get("engine", "SP"),
                                "ins": [], "outs": [],
                                "debug": inst.get("debug", 0),
                                "sync_info": {"on_wait": [w]},
                            })
                        si["on_wait"] = [waits[-1]]
                    if len(upds) > 1:
                        if inst.get("opcode") == "DMACopy":
                            raise AssertionError("multi-update DMACopy")
                        changed = True
                        extra = upds[1:]
                        si["on_update"] = [upds[0]]
                        out.append(inst)
                        for k, u in enumerate(extra):
                            out.append({
                                "name": f"{inst['name']}_pu{k}",
                                "opcode": "NoOp",
                                "engine": inst.get("engine", "SP"),
                                "ins": [], "outs": [],
                                "debug": inst.get("debug", 0),
                                "sync_info": {"on_update": [u]},
                            })
                        continue
                out.append(inst)
            blk["instructions"] = out
    return orjson.dumps(m) if changed else bir


def _install_bir_fix():
    global _bir_fix_installed
    if _bir_fix_installed:
        return
    _bir_fix_installed = True
    import concourse.bass_utils as bu
    import concourse.bass2jax as b2j

    orig = bu.compile_bir_kernel

    def patched(bir_json, tmpdir, neff_name="file.neff"):
        return orig(_fix_bir_bytes(bytes(bir_json)), tmpdir, neff_name)

    bu.compile_bir_kernel = patched
    b2j.compile_bir_kernel = patched


# ---------------------------------------------------------------------------
# Device kernel builder
# ---------------------------------------------------------------------------


def build_bass_kernel(C):
    """One expert's SwiGLU FFN over C token columns (zero-padded).

    DRAM inputs (per core):
      xt   [P, DK, C]         x.T tiled:      xt[p, dk, c]   = x[c, dk*128+p]
      wg   [FI, P, DK, 128]   Wg.T tiled:     wg[fi, p, dk, fj] = Wg[fi*128+fj, dk*128+p]
      w1   [FI, P, DK, 128]   same for W1
      w2   [DO, P, FI, 128]   W2.T tiled:     w2[do, p, fi, dj] = W2[do*128+dj, fi*128+p]
    DRAM output:
      outt [P, DO, C]         out.T tiled:    outt[p, do, c] = out[c, do*128+p]
    """
    import concourse.bass as bass
    import concourse.mybir as mybir
    import concourse.tile as tile

    f32 = mybir.dt.float32
    wdt = mybir.dt.bfloat16 if USE_BF16 else f32

    nc = bass.Bass("TRN2")
    xt_d = nc.dram_tensor("xt", [P, DK, C], wdt, kind="ExternalInput")
    wg_d = nc.dram_tensor("wg", [FI, P, DK, P], wdt, kind="ExternalInput")
    w1_d = nc.dram_tensor("w1", [FI, P, DK, P], wdt, kind="ExternalInput")
    w2_d = nc.dram_tensor("w2", [DO, P, FI, P], wdt, kind="ExternalInput")
    out_d = nc.dram_tensor("outt", [P, DO, C], f32, kind="ExternalOutput")

    t_tiles = []
    t0 = 0
    while t0 < C:
        tw = min(TW, C - t0)
        t_tiles.append((t0, tw))
        t0 += tw

    with tile.TileContext(nc) as tc:
        with (
            tc.tile_pool(name="xpool", bufs=2) as xpool,
            tc.tile_pool(name="wpool", bufs=3) as wpool,
            tc.tile_pool(name="w2pool", bufs=2) as w2pool,
            tc.tile_pool(name="zpool", bufs=1) as zpool,
            tc.tile_pool(name="gpool", bufs=3) as gpool,
            tc.tile_pool(name="opool", bufs=2) as opool,
            tc.tile_pool(name="psum", bufs=2, space="PSUM") as psum,
        ):
            for (t0, tw) in t_tiles:
                xt = xpool.tile([P, DK, TW], wdt, tag="xt")
                nc.sync.dma_start(xt[:, :, :tw], xt_d[:, :, t0:t0 + tw])
                z = zpool.tile([P, FI, TW], wdt, tag="z")
                for fi in range(FI):
                    wg = wpool.tile([P, DK, P], wdt, tag="wg")
                    nc.sync.dma_start(wg[:], wg_d[fi])
                    pg = psum.tile([P, TW], f32, tag="pg")
                    for dk in range(DK):
                        nc.tensor.matmul(
                            pg[:, :tw], wg[:, dk, :], xt[:, dk, :tw],
                            start=(dk == 0), stop=(dk == DK - 1),
                        )
                    w1 = wpool.tile([P, DK, P], wdt, tag="w1")
                    nc.sync.dma_start(w1[:], w1_d[fi])
                    pv = psum.tile([P, TW], f32, tag="pv")
                    for dk in range(DK):
                        nc.tensor.matmul(
                            pv[:, :tw], w1[:, dk, :], xt[:, dk, :tw],
                            start=(dk == 0), stop=(dk == DK - 1),
                        )
                    g = gpool.tile([P, TW], f32, tag="g")
                    nc.scalar.activation(
                        g[:, :tw], pg[:, :tw],
                        mybir.ActivationFunctionType.Silu,
                    )
                    nc.vector.tensor_mul(z[:, fi, :tw], g[:, :tw], pv[:, :tw])
                ot = opool.tile([P, DO, TW], f32, tag="ot")
                for do in range(DO):
                    w2 = w2pool.tile([P, FI, P], wdt, tag="w2")
                    nc.sync.dma_start(w2[:], w2_d[do])
                    po = psum.tile([P, TW], f32, tag="po")
                    for fi in range(FI):
                        nc.tensor.matmul(
                            po[:, :tw], w2[:, fi, :], z[:, fi, :tw],
                            start=(fi == 0), stop=(fi == FI - 1),
                        )
                    nc.vector.tensor_copy(ot[:, do, :tw], po[:, :tw])
                nc.sync.dma_start(out_d[:, :, t0:t0 + tw], ot[:, :, :tw])
    return nc


# ---------------------------------------------------------------------------
# Host wrapper
# ---------------------------------------------------------------------------


def _route(xt, gate_W, gate_b, k):
    """Return per-expert (token_idx, prob) using top-k + softmax-over-top-k."""
    logits = xt @ gate_W.T + gate_b  # [T, E]
    T, E = logits.shape
    # top-k indices, matching jax.lax.top_k (descending by value)
    idx = np.argpartition(-logits, k - 1, axis=1)[:, :k]
    vals = np.take_along_axis(logits, idx, axis=1)
    order = np.argsort(-vals, axis=1, kind="stable")
    idx = np.take_along_axis(idx, order, axis=1)
    vals = np.take_along_axis(vals, order, axis=1)
    vals = vals - vals.max(axis=1, keepdims=True)
    ex = np.exp(vals)
    probs = ex / ex.sum(axis=1, keepdims=True)  # [T, k]
    per_expert = []
    flat_e = idx.reshape(-1)
    flat_t = np.repeat(np.arange(T), k)
    flat_p = probs.reshape(-1)
    for e in range(E):
        m = flat_e == e
        per_expert.append((flat_t[m], flat_p[m]))
    return per_expert


def kernel(x, gate_W, gate_b, Wg, W1, W2, num_experts_per_token):
    _install_bir_fix()
    from concourse.bass_utils import run_bass_kernel_spmd
    import ml_dtypes

    x = np.asarray(x, dtype=np.float32)
    gate_W = np.asarray(gate_W, dtype=np.float32)
    gate_b = np.asarray(gate_b, dtype=np.float32)
    Wg = np.asarray(Wg, dtype=np.float32)
    W1 = np.asarray(W1, dtype=np.float32)
    W2 = np.asarray(W2, dtype=np.float32)
    k = int(num_experts_per_token)

    B, S, D = x.shape
    T = B * S
    xt = x.reshape(T, D)
    per_expert = _route(xt, gate_W, gate_b, k)

    maxN = max(len(t) for t, _ in per_expert)
    C = max(TW, -(-maxN // P) * P)  # round up to multiple of 128, >= 512

    wdt = ml_dtypes.bfloat16 if USE_BF16 else np.float32
    nc = build_bass_kernel(C)

    in_maps = []
    for e in range(N_EXPERTS):
        tok, _ = per_expert[e]
        n = len(tok)
        xe = np.zeros((P, DK, C), dtype=wdt)
        # xt[tok].T -> [D, n] -> [DK, P, n] -> [P, DK, n]
        xe[:, :, :n] = (
            xt[tok].T.reshape(DK, P, n).transpose(1, 0, 2).astype(wdt)
        )
        wg_e = np.ascontiguousarray(
            Wg[e].reshape(FI, P, DK, P).transpose(0, 3, 2, 1)
        ).astype(wdt)
        w1_e = np.ascontiguousarray(
            W1[e].reshape(FI, P, DK, P).transpose(0, 3, 2, 1)
        ).astype(wdt)
        w2_e = np.ascontiguousarray(
            W2[e].reshape(DO, P, FI, P).transpose(0, 3, 2, 1)
        ).astype(wdt)
        in_maps.append({"xt": xe, "wg": wg_e, "w1": w1_e, "w2": w2_e})

    res = run_bass_kernel_spmd(nc, in_maps, core_ids=list(range(N_CORES)))

    out = np.zeros((T, D), dtype=np.float32)
    for e in range(N_EXPERTS):
        tok, prob = per_expert[e]
        n = len(tok)
        oe = res.results[e]["outt"]  # [P, DO, C]
        oe = oe[:, :, :n].transpose(1, 0, 2).reshape(D, n).T  # [n, D]
        np.add.at(out, tok, oe * prob[:, None].astype(np.float32))
    return out.reshape(B, S, D)


# revision 3
# speedup vs baseline: 1.8989x; 1.8989x over previous
"""MoE ExpertFeedForward (top-k routing + per-expert SwiGLU) on 8 Trainium2
NeuronCores via expert parallelism.

Host: router (logits/top-k/softmax), token dispatch, weighted combine.
Device (one expert per core): X.T laid out [D, C] column-major tokens;
  G = silu(Wg @ x), V = W1 @ x, Z = G*V, out.T = W2 @ Z,
  all as 128x128-stationary matmuls streaming token columns.
"""

import numpy as np

D_MODEL = 1024
D_FF = 4096
N_EXPERTS = 8
N_CORES = 8
P = 128
TW = 512  # token-tile width (matmul moving free dim / one PSUM bank of fp32)
DK = D_MODEL // P   # 8 contraction tiles for MM1/MM2
FI = D_FF // P      # 32 f tiles
DO = D_MODEL // P   # 8 output-d tiles for MM3

USE_BF16 = True

# ---------------------------------------------------------------------------
# BIR post-pass: TRN2 instruction encoding has ONE sync-wait slot; Tile can
# emit several waits on one instruction, which this walrus build rejects
# ("Too many sync wait commands").  Peel extra waits onto single-wait NoOps
# inserted just before the instruction on the same engine.
# ---------------------------------------------------------------------------

_bir_fix_installed = False


def _fix_bir_bytes(bir):
    import orjson

    m = orjson.loads(bir)
    changed = False
    for fn in m.get("functions", []):
        for blk in fn.get("blocks", []):
            out = []
            for inst in blk.get("instructions", []):
                si = inst.get("sync_info")
                if si:
                    waits = si.get("on_wait") or []
                    upds = si.get("on_update") or []
                    if len(waits) > 1:
                        changed = True
                        for k, w in enumerate(waits[:-1]):
                            out.append({
                                "name": f"{inst['name']}_pw{k}",
                                "opcode": "NoOp",
                                "engine": inst.get("engine", "SP"),
                                "ins": [], "outs": [],
                                "debug": inst.get("debug", 0),
                                "sync_info": {"on_wait": [w]},
                            })
                        si["on_wait"] = [waits[-1]]
                    if len(upds) > 1:
                        if inst.get("opcode") == "DMACopy":
                            raise AssertionError("multi-update DMACopy")
                        changed = True
                        extra = upds[1:]
                        si["on_update"] = [upds[0]]
                        out.append(inst)
                        for k, u in enumerate(extra):
                            out.append({
                                "name": f"{inst['name']}_pu{k}",
                                "opcode": "NoOp",
                                "engine": inst.get("engine", "SP"),
                                "ins": [], "outs": [],
                                "debug": inst.get("debug", 0),
                                "sync_info": {"on_update": [u]},
                            })
                        continue
                out.append(inst)
            blk["instructions"] = out
    return orjson.dumps(m) if changed else bir


def _install_bir_fix():
    global _bir_fix_installed
    if _bir_fix_installed:
        return
    _bir_fix_installed = True
    import concourse.bass_utils as bu
    import concourse.bass2jax as b2j

    orig = bu.compile_bir_kernel

    def patched(bir_json, tmpdir, neff_name="file.neff"):
        return orig(_fix_bir_bytes(bytes(bir_json)), tmpdir, neff_name)

    bu.compile_bir_kernel = patched
    b2j.compile_bir_kernel = patched


# ---------------------------------------------------------------------------
# Device kernel builder
# ---------------------------------------------------------------------------


def build_bass_kernel(C, repeat=1):
    """One expert's SwiGLU FFN over C token columns (zero-padded).

    DRAM inputs (per core):
      xt   [P, DK, C]         x.T tiled:      xt[p, dk, c]   = x[c, dk*128+p]
      wg   [FI, P, DK, 128]   Wg.T tiled:     wg[fi, p, dk, fj] = Wg[fi*128+fj, dk*128+p]
      w1   [FI, P, DK, 128]   same for W1
      w2   [DO, P, FI, 128]   W2.T tiled:     w2[do, p, fi, dj] = W2[do*128+dj, fi*128+p]
    DRAM output:
      outt [P, DO, C]         out.T tiled:    outt[p, do, c] = out[c, do*128+p]
    """
    import concourse.bass as bass
    import concourse.mybir as mybir
    import concourse.tile as tile

    f32 = mybir.dt.float32
    wdt = mybir.dt.bfloat16 if USE_BF16 else f32

    nc = bass.Bass("TRN2")
    xt_d = nc.dram_tensor("xt", [P, DK, C], wdt, kind="ExternalInput")
    wg_d = nc.dram_tensor("wg", [FI, P, DK, P], wdt, kind="ExternalInput")
    w1_d = nc.dram_tensor("w1", [FI, P, DK, P], wdt, kind="ExternalInput")
    w2_d = nc.dram_tensor("w2", [DO, P, FI, P], wdt, kind="ExternalInput")
    out_d = nc.dram_tensor("outt", [P, DO, C], f32, kind="ExternalOutput")

    t_tiles = []
    t0 = 0
    while t0 < C:
        tw = min(TW, C - t0)
        t_tiles.append((t0, tw))
        t0 += tw

    import contextlib

    with tile.TileContext(nc) as tc:
        with (
            tc.tile_pool(name="xpool", bufs=2) as xpool,
            tc.tile_pool(name="wpool", bufs=3) as wpool,
            tc.tile_pool(name="w2pool", bufs=2) as w2pool,
            tc.tile_pool(name="zpool", bufs=1) as zpool,
            tc.tile_pool(name="gpool", bufs=3) as gpool,
            tc.tile_pool(name="opool", bufs=2) as opool,
            tc.tile_pool(name="psum", bufs=2, space="PSUM") as psum,
            tc.For_i(0, repeat, 1) if repeat > 1 else contextlib.nullcontext(),
        ):
            for (t0, tw) in t_tiles:
                xt = xpool.tile([P, DK, TW], wdt, tag="xt")
                nc.sync.dma_start(xt[:, :, :tw], xt_d[:, :, t0:t0 + tw])
                z = zpool.tile([P, FI, TW], wdt, tag="z")
                for fi in range(FI):
                    wg = wpool.tile([P, DK, P], wdt, tag="wg")
                    nc.sync.dma_start(wg[:], wg_d[fi])
                    pg = psum.tile([P, TW], f32, tag="pg")
                    for dk in range(DK):
                        nc.tensor.matmul(
                            pg[:, :tw], wg[:, dk, :], xt[:, dk, :tw],
                            start=(dk == 0), stop=(dk == DK - 1),
                        )
                    w1 = wpool.tile([P, DK, P], wdt, tag="w1")
                    nc.sync.dma_start(w1[:], w1_d[fi])
                    pv = psum.tile([P, TW], f32, tag="pv")
                    for dk in range(DK):
                        nc.tensor.matmul(
                            pv[:, :tw], w1[:, dk, :], xt[:, dk, :tw],
                            start=(dk == 0), stop=(dk == DK - 1),
                        )
                    g = gpool.tile([P, TW], f32, tag="g")
                    nc.scalar.activation(
                        g[:, :tw], pg[:, :tw],
                        mybir.ActivationFunctionType.Silu,
                    )
                    nc.vector.tensor_mul(z[:, fi, :tw], g[:, :tw], pv[:, :tw])
                ot = opool.tile([P, DO, TW], f32, tag="ot")
                for do in range(DO):
                    w2 = w2pool.tile([P, FI, P], wdt, tag="w2")
                    nc.sync.dma_start(w2[:], w2_d[do])
                    po = psum.tile([P, TW], f32, tag="po")
                    for fi in range(FI):
                        nc.tensor.matmul(
                            po[:, :tw], w2[:, fi, :], z[:, fi, :tw],
                            start=(fi == 0), stop=(fi == FI - 1),
                        )
                    nc.vector.tensor_copy(ot[:, do, :tw], po[:, :tw])
                nc.sync.dma_start(out_d[:, :, t0:t0 + tw], ot[:, :, :tw])
    return nc


# ---------------------------------------------------------------------------
# Host wrapper
# ---------------------------------------------------------------------------


def _route(xt, gate_W, gate_b, k):
    """Return per-expert (token_idx, prob) using top-k + softmax-over-top-k."""
    logits = xt @ gate_W.T + gate_b  # [T, E]
    T, E = logits.shape
    # top-k indices, matching jax.lax.top_k (descending by value)
    idx = np.argpartition(-logits, k - 1, axis=1)[:, :k]
    vals = np.take_along_axis(logits, idx, axis=1)
    order = np.argsort(-vals, axis=1, kind="stable")
    idx = np.take_along_axis(idx, order, axis=1)
    vals = np.take_along_axis(vals, order, axis=1)
    vals = vals - vals.max(axis=1, keepdims=True)
    ex = np.exp(vals)
    probs = ex / ex.sum(axis=1, keepdims=True)  # [T, k]
    per_expert = []
    flat_e = idx.reshape(-1)
    flat_t = np.repeat(np.arange(T), k)
    flat_p = probs.reshape(-1)
    for e in range(E):
        m = flat_e == e
        per_expert.append((flat_t[m], flat_p[m]))
    return per_expert


def kernel(x, gate_W, gate_b, Wg, W1, W2, num_experts_per_token):
    _install_bir_fix()
    from concourse.bass_utils import run_bass_kernel_spmd
    import ml_dtypes

    x = np.asarray(x, dtype=np.float32)
    gate_W = np.asarray(gate_W, dtype=np.float32)
    gate_b = np.asarray(gate_b, dtype=np.float32)
    Wg = np.asarray(Wg, dtype=np.float32)
    W1 = np.asarray(W1, dtype=np.float32)
    W2 = np.asarray(W2, dtype=np.float32)
    k = int(num_experts_per_token)

    B, S, D = x.shape
    T = B * S
    xt = x.reshape(T, D)
    per_expert = _route(xt, gate_W, gate_b, k)

    maxN = max(len(t) for t, _ in per_expert)
    C = max(TW, -(-maxN // P) * P)  # round up to multiple of 128, >= 512

    wdt = ml_dtypes.bfloat16 if USE_BF16 else np.float32
    nc = build_bass_kernel(C)

    in_maps = []
    for e in range(N_EXPERTS):
        tok, _ = per_expert[e]
        n = len(tok)
        xe = np.zeros((P, DK, C), dtype=wdt)
        # xt[tok].T -> [D, n] -> [DK, P, n] -> [P, DK, n]
        xe[:, :, :n] = (
            xt[tok].T.reshape(DK, P, n).transpose(1, 0, 2).astype(wdt)
        )
        wg_e = np.ascontiguousarray(
            Wg[e].reshape(FI, P, DK, P).transpose(0, 3, 2, 1)
        ).astype(wdt)
        w1_e = np.ascontiguousarray(
            W1[e].reshape(FI, P, DK, P).transpose(0, 3, 2, 1)
        ).astype(wdt)
        w2_e = np.ascontiguousarray(
            W2[e].reshape(DO, P, FI, P).transpose(0, 3, 2, 1)
        ).astype(wdt)
        in_maps.append({"xt": xe, "wg": wg_e, "w1": w1_e, "w2": w2_e})

    res = run_bass_kernel_spmd(nc, in_maps, core_ids=list(range(N_CORES)))

    out = np.zeros((T, D), dtype=np.float32)
    for e in range(N_EXPERTS):
        tok, prob = per_expert[e]
        n = len(tok)
        oe = res.results[e]["outt"]  # [P, DO, C]
        oe = oe[:, :, :n].transpose(1, 0, 2).reshape(D, n).T  # [n, D]
        np.add.at(out, tok, oe * prob[:, None].astype(np.float32))
    return out.reshape(B, S, D)
